# revision 20
# baseline (speedup 1.0000x reference)
"""Trainium2 Bass kernel for masked causal attention with RoPE (mgdt column masking).

v2: software-pipelined attention, stream_shuffle RoPE swap, ACT/DVE exp split
(exact exp + bf16 Schraudolph), bf16 norm path, PSUM ring plan 4+4 banks.

Sharding: 8 cores = data-parallel over batch (4) x tensor-parallel over head
groups (2 x 4 heads). Each core computes a [T, H] partial of its batch's
output projection (Wo row-sharded); host sums the pair of partials + bo.
"""

import sys

if "/opt/trn_rl_repo" not in sys.path:
    sys.path.insert(0, "/opt/trn_rl_repo")

import math
from collections import deque

import numpy as np
import ml_dtypes

B, T, H, NH, D = 4, 2048, 512, 8, 64
THETA = 10000.0
PERIOD, RET_ORDER = 3, 2
NCORES = 8
CPG = H // 2          # 256 channels per head-group shard
CHUNK = 512           # t-chunk (one PSUM bank of fp32)
NCH = T // CHUNK      # 4 query chunks
BF = ml_dtypes.bfloat16

WARMUP_MMS = 56
DVE_EXP_CAP = 0.5     # max fraction of exp width on the vector engine
A_SCHR = 0.125 * math.log2(math.e) * 128.0
B_SCHR = 127.0 * 128.0 - 6.0

# --- compacted key geometry (host + builder share this) ---
_cm = np.ones(T, bool)
_cm[PERIOD + RET_ORDER - 1::PERIOD] = False
POS = np.where(_cm)[0]              # 1366 unmasked key positions
NSC_RAW = len(POS)                  # 1366
NSTC = (NSC_RAW + 127) // 128       # 11 s-tiles
NSC = NSTC * 128                    # 1408 padded
KCW = [CHUNK, CHUNK, NSC - 2 * CHUNK]   # k-projection chunk widths (512,512,384)

_INF = 1 << 30
TILE_LO = [int(POS[128 * i]) if 128 * i < NSC_RAW else _INF for i in range(NSTC)]
TILE_HI = [int(POS[min(128 * i + 127, NSC_RAW - 1)]) if 128 * i < NSC_RAW else _INF
           for i in range(NSTC)]


def _tiles_for_chunk(j):
    """(i, col0, crossing) per compact s-tile of t-chunk j. col0 rounded even."""
    out = []
    for i in range(NSTC):
        lo, hi = TILE_LO[i], TILE_HI[i]
        if lo > CHUNK * j + CHUNK - 1:
            continue
        col0 = max(0, lo - CHUNK * j) & ~1
        crossing = hi > CHUNK * j
        out.append((i, col0, crossing))
    return out


_CROSSINGS = sorted({(i, j) for j in range(NCH)
                     for (i, c0, cr) in _tiles_for_chunk(j) if cr})

_prog = None


def _build_program():
    global _prog
    if _prog is not None:
        return _prog
    from contextlib import ExitStack
    import concourse.bacc as bacc
    import concourse.tile as tile
    from concourse import mybir
    from concourse.bass import AP as BassAP

    bf = mybir.dt.bfloat16
    f32 = mybir.dt.float32
    i16 = mybir.dt.int16
    EXP = mybir.ActivationFunctionType.Exp
    IDENT = mybir.ActivationFunctionType.Identity
    MULT = mybir.AluOpType.mult
    ADD = mybir.AluOpType.add

    nc = bacc.Bacc("TRN2", target_bir_lowering=False, debug=False, num_devices=NCORES)

    def din(name, shape, dt):
        return nc.dram_tensor(name, shape, dt, kind="ExternalInput").ap()

    nm = len(_CROSSINGS)
    xt_d = din("xt", [128, 4 * T], bf)            # x^T 4 row-tiles side by side
    xtc_d = din("xtc", [128, 4 * NSC], bf)        # compacted x^T row-tiles
    xtc1_d = din("xtc1", [1, NSC], bf)            # ones row (zero at pad cols)
    wqall_d = din("wqall", [128, 2 * CPG * 4], bf)  # wq|wk, each 4 kt x 256
    wv_d = din("wv", [128, 4 * 260], bf)
    wvb_d = din("wvb", [1, 260], bf)
    wo_d = din("wo", [128, 2 * H], bf)
    trig_d = din("trig", [128, 2 * (2 * T + 2 * NSC)], bf)  # cos|sin|cosk|sink
    bm_d = din("bmask", [128, nm * CHUNK], bf)
    bias_d = din("biases", [128, 4], f32)         # bq ct0|bq ct1|bk ct0|bk ct1
    selm_d = din("selm", [128, 256], mybir.dt.float32r)  # row 64: [1x64 0x64 | 0x64 1x64]
    out_d = nc.dram_tensor("out", [T, H], f32, kind="ExternalOutput").ap()

    TRIG_COS, TRIG_SIN = 0, 2 * T
    TRIG_COSK, TRIG_SINK = 4 * T, 4 * T + 2 * NSC
    XOR1 = [i ^ 1 for i in range(32)]

    # build-time engine load balancing (estimated ns)
    load = {"act": 0.0, "dve": 0.0, "gps": 0.0}

    def pick(costs):
        eng = min(costs, key=lambda e: load[e] + costs[e])
        load[eng] += costs[eng]
        return eng

    def c_act(w):
        return w * 1.042 + 300.0

    def c_dve(w):
        return w * 1.042 + 170.0

    with tile.TileContext(nc) as tc:
        with ExitStack() as ctx:
            sg = ctx.enter_context(tc.tile_pool(name="sg", bufs=1))

            def ld(name, dram, cols, splits, dt=bf):
                tl = sg.tile([128, cols], dt, tag=name, name=name)
                for a, b in splits:
                    nc.sync.dma_start(out=tl[:, a:b], in_=dram[:, a:b])
                return tl

            def spl(cols, n):
                step = -(-cols // n)
                return [(a, min(a + step, cols)) for a in range(0, cols, step)]

            # ---- loads, in first-use order ----
            wqall = ld("wqall", wqall_d, 2 * CPG * 4, spl(2048, 2))
            xtcall = ld("xtcall", xtc_d, 4 * NSC, spl(4 * NSC, 4))
            trig = sg.tile([128, 2 * (2 * T + 2 * NSC)], bf, tag="trig", name="trig")
            for a, b in spl(2 * 2 * NSC, 2):      # cosk|sink first (K rope)
                nc.sync.dma_start(out=trig[:, 4 * T + a:4 * T + b],
                                  in_=trig_d[:, 4 * T + a:4 * T + b])
            wvall = ld("wvall", wv_d, 4 * 260, [(0, 4 * 260)])
            wv_b = sg.tile([1, 260], bf, tag="wvb")
            nc.sync.dma_start(out=wv_b, in_=wvb_d[:, :])
            xtc1 = sg.tile([1, NSC], bf, tag="xtc1")
            nc.sync.dma_start(out=xtc1, in_=xtc1_d[:, :])
            bias_sb = sg.tile([128, 4], f32, tag="biases")
            nc.sync.dma_start(out=bias_sb, in_=bias_d[:, :])
            xtall = ld("xtall", xt_d, 4 * T, spl(4 * T, 4))
            for a, b in spl(4 * T, 4):            # cos|sin for Q rope
                nc.sync.dma_start(out=trig[:, a:b], in_=trig_d[:, a:b])
            bmall = ld("bmall", bm_d, nm * CHUNK, spl(nm * CHUNK, 2))
            woall = ld("woall", wo_d, 2 * H, [(0, 2 * H)])

            def xts(kt, csl):
                return xtall[:, 2048 * kt + csl.start: 2048 * kt + csl.stop]

            def xtcs(kt, csl):
                return xtcall[:, NSC * kt + csl.start: NSC * kt + csl.stop]

            def wslc(v, kt, ct):
                base = 1024 * v + 256 * kt + 128 * ct
                return wqall[:, base:base + 128]

            def bm_slice(i, j, c0, w2):
                n = _CROSSINGS.index((i, j))
                return bmall[:, CHUNK * n + c0:CHUNK * n + c0 + w2]

            # persistent activations
            krot = {}
            for ct in range(2):
                for kc in range(3):
                    krot[ct, kc] = sg.tile([128, CHUNK], bf, tag=f"kr{ct}_{kc}",
                                           name=f"kr{ct}_{kc}")
            vaug = []
            for s in range(NSTC):
                vaug.append(sg.tile([128, 260], bf, tag=f"va{s}", name=f"va{s}"))

            qp = ctx.enter_context(tc.tile_pool(name="qp", bufs=4))
            rtmp = ctx.enter_context(tc.tile_pool(name="rtmp", bufs=2))
            pp = ctx.enter_context(tc.tile_pool(name="pp", bufs=8))
            stg = ctx.enter_context(tc.tile_pool(name="stg", bufs=3))
            dn = ctx.enter_context(tc.tile_pool(name="dn", bufs=2))
            aotp = ctx.enter_context(tc.tile_pool(name="aotp", bufs=4))
            ost = ctx.enter_context(tc.tile_pool(name="ost", bufs=3))

            qrot = {}

            def rope_call(pool, v, ct, dst, xsl, cos_off, sin_off, w, tag):
                """dst <- rope(x @ W + b) for one chunk of width w."""
                if pool.name == "ppj":
                    pq = pool.tile([128, CHUNK], f32, tag="ps", name=f"pq_{tag}")
                else:
                    pq = pool.tile([128, CHUNK], f32, tag="pq", name=f"pq_{tag}",
                                   bufs=1)
                for kt in range(4):
                    nc.tensor.matmul(pq[:, :w], lhsT=wslc(v, kt, ct), rhs=xsl(kt),
                                     start=(kt == 0), stop=(kt == 3))
                qs = rtmp.tile([128, CHUNK], bf, tag="qs", name=f"qs_{tag}")
                load["act"] += c_act(w)
                nc.scalar.activation(out=qs[:, :w], in_=pq[:, :w], func=IDENT,
                                     bias=bias_sb[:, 2 * v + ct:2 * v + ct + 1],
                                     scale=1.0)
                qw = rtmp.tile([128, CHUNK], bf, tag="qw", name=f"qw_{tag}")
                load["dve"] += 3 * c_dve(w)
                nc.vector.stream_shuffle(out=qw[:, :w], in_=qs[:, :w], mask=XOR1)
                t1 = rtmp.tile([128, CHUNK], bf, tag="t1", name=f"t1_{tag}")
                nc.vector.tensor_mul(t1[:, :w], qs[:, :w],
                                     trig[:, cos_off:cos_off + w])
                t2 = rtmp.tile([128, CHUNK], bf, tag="t2", name=f"t2_{tag}")
                nc.vector.tensor_mul(t2[:, :w], qw[:, :w],
                                     trig[:, sin_off:sin_off + w])
                load["gps"] += w * 2.9 + 550.0
                nc.gpsimd.tensor_add(dst[:, :w], t1[:, :w], t2[:, :w])

            def q_rope(pool, ct, j):
                dst = qp.tile([128, CHUNK], bf, tag="qr", name=f"qr{ct}_{j}")
                qrot[ct, j] = dst
                rope_call(pool, 0, ct, dst,
                          lambda kt: xts(kt, slice(CHUNK * j, CHUNK * (j + 1))),
                          TRIG_COS + T * ct + CHUNK * j,
                          TRIG_SIN + T * ct + CHUNK * j, CHUNK, f"q{ct}_{j}")

            # ---- phase B: warmup + K/V projections + Q rope for j=3 ----
            with tc.tile_pool(name="ppj", bufs=4, space="PSUM") as ppj:
                if WARMUP_MMS:
                    wt = sg.tile([128, 128], bf, tag="warm")
                    nc.gpsimd.memset(wt, 0.0)
                    pwu = ppj.tile([128, CHUNK], f32, tag="ps", name="pwu")
                    for _ in range(WARMUP_MMS):
                        nc.tensor.matmul(pwu[:, 0:128], lhsT=wt, rhs=wt,
                                         start=True, stop=True)
                for ct in range(2):
                    for kc, w in enumerate(KCW):
                        rope_call(
                            ppj, 1, ct, krot[ct, kc],
                            lambda kt: xtcs(kt, slice(CHUNK * kc, CHUNK * kc + w)),
                            TRIG_COSK + NSC * ct + CHUNK * kc,
                            TRIG_SINK + NSC * ct + CHUNK * kc, w, f"k{ct}_{kc}")
                for s in range(NSTC):
                    ssl = slice(128 * s, 128 * (s + 1))
                    pv = ppj.tile([128, CHUNK], f32, tag="ps", name=f"pv{s}")
                    for kt in range(4):
                        nc.tensor.matmul(pv[:, 0:260], lhsT=xtcs(kt, ssl),
                                         rhs=wvall[:, 260 * kt:260 * (kt + 1)],
                                         start=(kt == 0), stop=False)
                    nc.tensor.matmul(pv[:, 0:260], lhsT=xtc1[0:1, ssl], rhs=wv_b,
                                     start=False, stop=True)
                    if pick({"act": c_act(260), "dve": c_dve(260)}) == "act":
                        nc.scalar.copy(out=vaug[s], in_=pv[:, 0:260])
                    else:
                        nc.vector.tensor_copy(out=vaug[s], in_=pv[:, 0:260])
                for ct in range(2):
                    q_rope(ppj, ct, 3)

            pps = ctx.enter_context(tc.tile_pool(name="pps", bufs=2, space="PSUM"))
            ring = ctx.enter_context(tc.tile_pool(name="ring", bufs=2, space="PSUM"))

            # ---- attention: flat task stream, po deferred by 2 tasks ----
            expw_tot = [0.0]
            expw_dve = [0.0]
            po_map = {}
            aot_tiles = {}
            pending = deque()

            def emit_task(j, hp, si, s, col0, crossing, first, last):
                ps = pps.tile([128, 2 * CHUNK], f32, tag="ps",
                              name=f"ps{j}_{hp}_{s}")
                for idx in range(2):
                    pb = 64 * idx
                    ksl = slice(128 * (s % 4), 128 * (s % 4) + 128)
                    nc.tensor.matmul(
                        ps[:, CHUNK * idx + col0:CHUNK * (idx + 1)],
                        lhsT=krot[hp, s // 4][pb:pb + 64, ksl],
                        rhs=qrot[hp, j][pb:pb + 64, col0:],
                        start=True, stop=True)
                pt = pp.tile([128, 2 * CHUNK], bf, tag="p", name=f"pt{j}_{hp}_{s}")
                w = 2 * CHUNK - col0
                expw_tot[0] += w
                dve_ok = (expw_dve[0] + w) <= DVE_EXP_CAP * expw_tot[0]
                costs = {"act": c_act(w)}
                if dve_ok:
                    costs["dve"] = c_dve(w)
                if pick(costs) == "act":
                    nc.scalar.activation(out=pt[:, col0:], in_=ps[:, col0:],
                                         func=EXP, scale=0.125)
                else:
                    expw_dve[0] += w
                    nc.vector.tensor_scalar(
                        out=pt[:, col0:].bitcast(i16), in0=ps[:, col0:],
                        scalar1=A_SCHR, scalar2=B_SCHR, op0=MULT, op1=ADD)
                if crossing:
                    w2 = CHUNK - col0
                    cost_d = 2 * (w2 * 1.042 + 170.0)
                    cost_g = 2 * w2 * 1.45 + 550.0
                    if pick({"dve": cost_d, "gps": cost_g}) == "dve":
                        bmb = bm_slice(s, j, col0, w2)
                        for idx in range(2):
                            sl = pt[:, CHUNK * idx + col0:CHUNK * (idx + 1)]
                            nc.vector.tensor_mul(sl, sl, bmb)
                    else:
                        base = pt[:, col0:col0 + w2]
                        pt2 = BassAP(tensor=base.tensor, offset=base.offset,
                                     ap=[base.ap[0], [CHUNK, 2], [1, w2]])
                        bmb = bm_slice(s, j, col0, w2)
                        bm2 = BassAP(tensor=bmb.tensor, offset=bmb.offset,
                                     ap=[bmb.ap[0], [0, 2], [1, w2]])
                        nc.gpsimd.tensor_mul(pt2, pt2, bm2)
                pending.append((j, hp, s, col0, first, last, pt))

            f32r = mybir.dt.float32r
            selm = sg.tile([128, 256], f32r, tag="selm")
            nc.sync.dma_start(out=selm, in_=selm_d[:, :])

            def norm(j, hp):
                po = po_map.pop((j, hp))
                oA = stg.tile([65, CHUNK], f32r, tag="oA", name=f"oA{j}_{hp}")
                oB = stg.tile([65, CHUNK], f32r, tag="oB", name=f"oB{j}_{hp}")
                for o, p in ((oA, po[0]), (oB, po[1])):
                    if pick({"act": c_act(CHUNK), "dve": c_dve(CHUNK)}) == "act":
                        nc.scalar.copy(out=o, in_=p)
                    else:
                        nc.vector.tensor_copy(out=o, in_=p)
                shb = stg.tile([128, CHUNK], f32r, tag="shb", name=f"shb{j}_{hp}",
                               bufs=2)
                nc.sync.dma_start(out=shb[64:128, :], in_=oB[0:64, :])
                # broadcast the two denominator rows across partitions (K=1
                # matmuls, f32r), then one full-width approx reciprocal.
                prd = ring.tile([128, CHUNK], f32, tag="prd",
                                name=f"prd{j}_{hp}", bufs=1)
                nc.tensor.matmul(prd, lhsT=selm[64:65, 0:128],
                                 rhs=oA[64:65, :], start=True, stop=False)
                nc.tensor.matmul(prd, lhsT=selm[64:65, 128:256],
                                 rhs=oB[64:65, :], start=False, stop=True)
                rpr = dn.tile([128, CHUNK], f32, tag="rprd", name=f"rp{j}_{hp}")
                load["dve"] += c_dve(CHUNK) + 2 * c_dve(CHUNK)
                nc.vector.reciprocal_approx_fast(out=rpr, in_=prd)
                aot_t = aotp.tile([128, CHUNK], bf, tag="aot", name=f"ao{hp}_{j}")
                aot_tiles[hp, j] = aot_t
                nc.vector.tensor_mul(aot_t[0:64, :], oA[0:64, :], rpr[0:64, :])
                nc.vector.tensor_mul(aot_t[64:128, :], shb[64:128, :],
                                     rpr[64:128, :])

            def outproj(j):
                for tt in range(4):
                    pout = ring.tile([128, H], f32, tag="po",
                                     name=f"pout{j}_{tt}")
                    for ct2 in range(2):
                        nc.tensor.matmul(
                            pout,
                            lhsT=aot_tiles[ct2, j][:, 128 * tt:128 * (tt + 1)],
                            rhs=woall[:, H * ct2:H * (ct2 + 1)],
                            start=(ct2 == 0), stop=(ct2 == 1))
                    osb = ost.tile([128, H], f32, tag="ost", name=f"osb{j}_{tt}")
                    if pick({"act": c_act(H), "dve": c_dve(H)}) == "act":
                        nc.scalar.copy(out=osb, in_=pout)
                    else:
                        nc.vector.tensor_copy(out=osb, in_=pout)
                    nc.sync.dma_start(
                        out=out_d[CHUNK * j + 128 * tt:CHUNK * j + 128 * (tt + 1), :],
                        in_=osb)

            def drain_one():
                j, hp, s, col0, first, last, pt = pending.popleft()
                if (j, hp) not in po_map:
                    po_map[j, hp] = [
                        ring.tile([65, CHUNK], f32, tag="po",
                                  name=f"po{j}_{hp}_{i}") for i in range(2)]
                po = po_map[j, hp]
                for idx in range(2):
                    hh = 2 * hp + idx
                    nc.tensor.matmul(
                        po[idx][:, col0:],
                        lhsT=vaug[s][:, 65 * hh:65 * hh + 65],
                        rhs=pt[:, CHUNK * idx + col0:CHUNK * (idx + 1)],
                        start=first, stop=last, skip_group_check=True)
                if last:
                    norm(j, hp)
                    if hp == 1:
                        outproj_q.append([j, 2])

            outproj_q = []

            def tick_outproj():
                for item in list(outproj_q):
                    item[1] -= 1
                    if item[1] <= 0:
                        outproj(item[0])
                        outproj_q.remove(item)

            for j in reversed(range(NCH)):
                tiles_j = _tiles_for_chunk(j)
                for hp in range(2):
                    for si, (s, col0, crossing) in enumerate(tiles_j):
                        emit_task(j, hp, si, s, col0, crossing,
                                  si == 0, si == len(tiles_j) - 1)
                        tick_outproj()
                        if hp == 0 and si == 1 and j > 0:
                            for ct in range(2):
                                q_rope(ring, ct, j - 1)
                        while len(pending) > 3:
                            drain_one()
                while pending:
                    drain_one()
            for item in outproj_q:
                outproj(item[0])

    nc.compile()
    _prog = nc
    return nc


def _host_inputs(x, Wq, bq, Wk, bk, Wv, bv, Wo, bo):
    """Build the 8 per-core input maps (packed mega-tensors, hardcoded shapes)."""
    x = np.asarray(x, np.float32)
    Wq, bq = np.asarray(Wq, np.float32), np.asarray(bq, np.float32)
    Wk, bk = np.asarray(Wk, np.float32), np.asarray(bk, np.float32)
    Wv, bv = np.asarray(Wv, np.float32), np.asarray(bv, np.float32)
    Wo = np.asarray(Wo, np.float32)

    def rowpack(a, cols):
        r = a.shape[0] // 128
        return np.concatenate([a[128 * i:128 * (i + 1)] for i in range(r)], axis=1)

    xt_all, xtc_all = [], []
    for b in range(B):
        xt = np.ascontiguousarray(x[b].T)            # (512, 2048)
        xtc = np.zeros((H, NSC), np.float32)
        xtc[:, :NSC_RAW] = xt[:, POS]
        xt_all.append(rowpack(xt, T).astype(BF))
        xtc_all.append(rowpack(xtc, NSC).astype(BF))
    ones_c = np.zeros((1, NSC), np.float32)
    ones_c[0, :NSC_RAW] = 1.0
    xtc1 = ones_c.astype(BF)

    inv = (1.0 / (THETA ** (np.arange(0, H, 2, dtype=np.float32) / H))).astype(np.float32)
    tpos = np.arange(T, dtype=np.float32)
    ang = tpos[:, None] * inv[None, :]
    cosf = np.cos(ang).astype(np.float32).T     # (256, T)
    sinf = np.sin(ang).astype(np.float32).T

    per_g = []
    for g in range(2):
        cols = slice(CPG * g, CPG * (g + 1))
        wq_g, wk_g = Wq[:, cols], Wk[:, cols]
        wv_a = np.zeros((H, 260), np.float32)
        wv_row = np.zeros((1, 260), np.float32)
        for hh in range(4):
            wv_a[:, 65 * hh:65 * hh + 64] = Wv[:, CPG * g + 64 * hh:CPG * g + 64 * (hh + 1)]
            wv_row[0, 65 * hh:65 * hh + 64] = bv[CPG * g + 64 * hh:CPG * g + 64 * (hh + 1)]
            wv_row[0, 65 * hh + 64] = 1.0
        wq4 = np.concatenate([rowpack(w, CPG) for w in (wq_g, wk_g)], axis=1)
        pr = slice(128 * g, 128 * (g + 1))
        cos_g = np.repeat(cosf[pr], 2, axis=0)
        sin_g = np.repeat(sinf[pr], 2, axis=0).copy()
        sin_g[0::2] *= -1.0
        cosk_g = np.zeros((CPG, NSC), np.float32)
        sink_g = np.zeros((CPG, NSC), np.float32)
        cosk_g[:, :NSC_RAW] = cos_g[:, POS]
        sink_g[:, :NSC_RAW] = sin_g[:, POS]
        trig = np.concatenate([rowpack(cos_g, T), rowpack(sin_g, T),
                               rowpack(cosk_g, NSC), rowpack(sink_g, NSC)], axis=1)
        biases = np.stack([
            bq[cols][:128], bq[cols][128:],
            bk[cols][:128], bk[cols][128:],
        ], axis=1).astype(np.float32)
        per_g.append(dict(
            wqall=wq4.astype(BF),
            wv=rowpack(wv_a, 260).astype(BF), wvb=wv_row.astype(BF),
            wo=rowpack(Wo[cols, :], H).astype(BF),
            trig=trig.astype(BF), biases=biases,
        ))

    spos = np.full(NSC, _INF, np.int64)
    spos[:NSC_RAW] = POS
    bmask = np.zeros((128, len(_CROSSINGS) * CHUNK), np.float32)
    for n, (i, j) in enumerate(_CROSSINGS):
        rows = spos[128 * i:128 * (i + 1)]
        tcols = np.arange(CHUNK * j, CHUNK * (j + 1))
        bmask[:, CHUNK * n:CHUNK * (n + 1)] = (rows[:, None] <= tcols[None, :])

    selm = np.zeros((128, 256), np.float32)
    selm[64, 0:64] = 1.0
    selm[64, 128 + 64:128 + 128] = 1.0
    shared = dict(bmask=bmask.astype(BF), xtc1=xtc1, selm=selm)
    in_maps = []
    for c in range(NCORES):
        b, g = c // 2, c % 2
        m = dict(xt=xt_all[b], xtc=xtc_all[b], **shared)
        m.update(per_g[g])
        in_maps.append(m)
    return in_maps


def run(inputs, trace=False):
    """Build+run; returns BassKernelResults (per-core partials in .results)."""
    from concourse.bass_utils import run_bass_kernel_spmd
    nc = _build_program()
    in_maps = _host_inputs(**inputs)
    res = run_bass_kernel_spmd(nc, in_maps, list(range(NCORES)), trace=trace)
    return res


def kernel(x, Wq, bq, Wk, bk, Wv, bv, Wo, bo):
    res = run(dict(x=x, Wq=Wq, bq=bq, Wk=Wk, bk=bk, Wv=Wv, bv=bv, Wo=Wo, bo=bo))
    bo = np.asarray(bo, np.float32)
    out = np.empty((B, T, H), np.float32)
    for b in range(B):
        out[b] = res.results[2 * b]["out"] + res.results[2 * b + 1]["out"] + bo[None, :]
    return out
